# revision 27
# baseline (speedup 1.0000x reference)
"""LongcatFlash MoE kernel for 8 Trainium2 NeuronCores.

Strategy (M1, dense expert-parallel):
  - Each core owns one routed expert (expert-parallel; sharding_hint).
  - The router (softmax + biased top-2 over 12 experts) is computed on every
    core, over all T tokens.  Router weights are PERMUTED per core so that the
    core's own expert is always column 0 (softmax/top-k are permutation
    invariant), letting one SPMD program serve all cores.
  - Each core runs its expert's SiLU-gated MLP densely over all T tokens in
    transposed layout (features on partitions, tokens on free dim), scales by
    its expert's gate (x 2.5 routed scale), and core 0 additionally adds the
    zero-expert (identity) contribution.
  - Host sums the 8 partial [H, T] outputs and transposes -> [T, H].
"""

import sys

import numpy as np

sys.path.insert(0, "/opt/trn_rl_repo")

import ml_dtypes  # noqa: E402

import concourse.bass as bass  # noqa: E402
from concourse import bacc, mybir  # noqa: E402
from concourse.bass_utils import run_bass_kernel_spmd  # noqa: E402
from concourse.tile import TileContext  # noqa: E402

T, H, I = 8192, 1024, 512
E, Z, K = 8, 4, 2
SCALE = 2.5
NCORES = 8
NE = E + Z  # 12 router columns
I2 = 2 * I  # 1024

F32 = mybir.dt.float32
BF16 = mybir.dt.bfloat16

NT = 512  # tokens per chunk
NCHUNK = T // NT


def _build_dense_program() -> bass.Bass:
    nc = bacc.Bacc()

    xT_d = nc.declare_dram_parameter("xT", [H, T], BF16, isOutput=False)
    w13T_d = nc.declare_dram_parameter("w13T", [H, I2], BF16, isOutput=False)
    w2T_d = nc.declare_dram_parameter("w2T", [I, H], BF16, isOutput=False)
    rwT_d = nc.declare_dram_parameter("rwT", [H, NE], BF16, isOutput=False)
    cbias_d = nc.declare_dram_parameter("cbias", [1, NE], F32, isOutput=False)
    ident_d = nc.declare_dram_parameter("ident", [128, 128], F32, isOutput=False)
    selg_d = nc.declare_dram_parameter("selg", [16, 128], F32, isOutput=False)
    selz_d = nc.declare_dram_parameter("selz", [16, 128], F32, isOutput=False)
    ones_d = nc.declare_dram_parameter("ones1", [1, 128], F32, isOutput=False)
    outT_d = nc.declare_dram_parameter("outT", [H, T], F32, isOutput=True)

    KH = H // 128  # 8 contraction tiles for H
    KI = I // 128  # 4 contraction tiles for I

    with TileContext(nc) as tc:
        with (
            tc.tile_pool(name="const", bufs=1) as cp,
            tc.tile_pool(name="router", bufs=2) as rp,
            tc.tile_pool(name="mlp", bufs=2) as mp,
            tc.tile_pool(name="outp", bufs=3) as op_,
            tc.tile_pool(name="psum", bufs=2, space="PSUM") as pp,
            tc.tile_pool(name="psum_r", bufs=1, space="PSUM") as ppr,
        ):
            # ---- preload everything ----
            xTs = []
            for k in range(KH):
                t = cp.tile([128, T], BF16, tag=f"xT{k}")
                nc.sync.dma_start(out=t, in_=xT_d[k * 128 : (k + 1) * 128, :])
                xTs.append(t)
            w13Ts = []
            for k in range(KH):
                t = cp.tile([128, I2], BF16, tag=f"w13T{k}")
                nc.sync.dma_start(out=t, in_=w13T_d[k * 128 : (k + 1) * 128, :])
                w13Ts.append(t)
            w2Ts = []
            for k in range(KI):
                t = cp.tile([128, H], BF16, tag=f"w2T{k}")
                nc.sync.dma_start(out=t, in_=w2T_d[k * 128 : (k + 1) * 128, :])
                w2Ts.append(t)
            rwTs = []
            for k in range(KH):
                t = cp.tile([128, NE], BF16, tag=f"rwT{k}")
                nc.sync.dma_start(out=t, in_=rwT_d[k * 128 : (k + 1) * 128, :])
                rwTs.append(t)
            cbias = cp.tile([1, NE], F32, tag="cbias")
            nc.sync.dma_start(out=cbias, in_=cbias_d[:, :])
            ident = cp.tile([128, 128], F32, tag="ident")
            nc.sync.dma_start(out=ident, in_=ident_d[:, :])
            ones1 = cp.tile([1, 128], F32, tag="ones1")
            nc.sync.dma_start(out=ones1, in_=ones_d[:, :])
            # row selectors: pick row 0 (gate) / row 12 (zero scale) of gzT
            sel_g = cp.tile([16, 128], F32, tag="sel_g")
            nc.sync.dma_start(out=sel_g, in_=selg_d[:, :])
            sel_z = cp.tile([16, 128], F32, tag="sel_z")
            nc.sync.dma_start(out=sel_z, in_=selz_d[:, :])
            # replicate correction bias across all 128 partitions once
            cbias_r = cp.tile([128, NE], F32, tag="cbias_r")
            ps_cb = ppr.tile([128, NE], F32, tag="psr")
            nc.tensor.matmul(out=ps_cb, lhsT=ones1, rhs=cbias, start=True, stop=True)
            nc.vector.tensor_copy(cbias_r, ps_cb)

            for c in range(NCHUNK):
                c0 = c * NT
                # ---------------- router for this chunk ----------------
                # gzT rows: 0 = own-expert gate (*2.5), 12 = zero-expert scale
                gzT = rp.tile([16, NT], F32, tag="gzT")
                for st in range(NT // 128):
                    t0 = c0 + st * 128
                    ps_log = ppr.tile([128, NE], F32, tag="psr")
                    for k in range(KH):
                        nc.tensor.matmul(
                            out=ps_log,
                            lhsT=xTs[k][:, t0 : t0 + 128],
                            rhs=rwTs[k],
                            start=(k == 0),
                            stop=(k == KH - 1),
                        )
                    logit = rp.tile([128, NE], F32, tag="logit")
                    nc.vector.tensor_copy(logit, ps_log)
                    mx = rp.tile([128, 1], F32, tag="mx")
                    nc.vector.tensor_reduce(
                        out=mx, in_=logit, axis=mybir.AxisListType.X,
                        op=mybir.AluOpType.max,
                    )
                    nmx = rp.tile([128, 1], F32, tag="nmx")
                    nc.vector.tensor_scalar_mul(nmx, mx, -1.0)
                    ex = rp.tile([128, NE], F32, tag="ex")
                    ssum = rp.tile([128, 1], F32, tag="ssum")
                    nc.scalar.activation(
                        out=ex, in_=logit, func=mybir.ActivationFunctionType.Exp,
                        bias=nmx[:, 0:1], accum_out=ssum,
                    )
                    rinv = rp.tile([128, 1], F32, tag="rinv")
                    nc.vector.reciprocal(rinv, ssum)
                    scores = rp.tile([128, NE], F32, tag="scores")
                    nc.vector.tensor_scalar_mul(scores, ex, rinv[:, 0:1])
                    sel = rp.tile([128, NE], F32, tag="sel")
                    nc.vector.tensor_tensor(
                        out=sel, in0=scores, in1=cbias_r, op=mybir.AluOpType.add,
                    )
                    top8 = rp.tile([128, 8], F32, tag="top8")
                    nc.vector.max(out=top8, in_=sel)
                    mask = rp.tile([128, NE], F32, tag="mask")
                    nc.vector.tensor_scalar(
                        out=mask, in0=sel, scalar1=top8[:, 1:2], scalar2=None,
                        op0=mybir.AluOpType.is_ge,
                    )
                    gz = rp.tile([128, 16], F32, tag="gz")
                    nc.vector.tensor_tensor(
                        out=gz[:, 0:NE], in0=scores, in1=mask,
                        op=mybir.AluOpType.mult,
                    )
                    nc.vector.tensor_scalar_mul(gz[:, 0:E], gz[:, 0:E], SCALE)
                    nc.vector.tensor_reduce(
                        out=gz[:, 12:13], in_=gz[:, E:NE],
                        axis=mybir.AxisListType.X, op=mybir.AluOpType.add,
                    )
                    nc.vector.tensor_scalar_mul(gz[:, 13:16], gz[:, 0:3], 0.0)
                    ps_t = ppr.tile([16, 128], F32, tag="psr")
                    nc.tensor.transpose(out=ps_t, in_=gz[:, 0:16], identity=ident)
                    nc.vector.tensor_copy(gzT[:, st * 128 : (st + 1) * 128], ps_t)
                # replicate gate/zero rows across all 128 partitions
                # (sel_z row 12 carries the core-0 flag, so the zero-expert
                # contribution is zeroed on cores 1..7 by the matmul itself)
                grep_sb = rp.tile([128, NT], F32, tag="grep")
                zrep_sb = rp.tile([128, NT], F32, tag="zrep")
                ps_rep = ppr.tile([128, NT], F32, tag="psrep")
                nc.tensor.matmul(
                    out=ps_rep, lhsT=sel_g, rhs=gzT, start=True, stop=True,
                )
                nc.vector.tensor_copy(grep_sb, ps_rep)
                ps_rep2 = ppr.tile([128, NT], F32, tag="psrep")
                nc.tensor.matmul(
                    out=ps_rep2, lhsT=sel_z, rhs=gzT, start=True, stop=True,
                )
                nc.vector.tensor_copy(zrep_sb, ps_rep2)

                # ---------------- expert MLP for this chunk ----------------
                actTs = []
                for pair in range(KI):
                    ps_g = pp.tile([128, NT], F32, tag="psg")
                    ps_u = pp.tile([128, NT], F32, tag="psu")
                    for k in range(KH):
                        nc.tensor.matmul(
                            out=ps_g,
                            lhsT=w13Ts[k][:, pair * 128 : (pair + 1) * 128],
                            rhs=xTs[k][:, c0 : c0 + NT],
                            start=(k == 0), stop=(k == KH - 1),
                        )
                    for k in range(KH):
                        nc.tensor.matmul(
                            out=ps_u,
                            lhsT=w13Ts[k][:, (pair + 4) * 128 : (pair + 5) * 128],
                            rhs=xTs[k][:, c0 : c0 + NT],
                            start=(k == 0), stop=(k == KH - 1),
                        )
                    sg = mp.tile([128, NT], F32, tag="sg")
                    nc.scalar.activation(
                        out=sg, in_=ps_g, func=mybir.ActivationFunctionType.Silu,
                    )
                    # tiny DVE op whose only job is to absorb the ACT->DVE
                    # semaphore wait: the following TensorTensor can then
                    # carry a single wait (PE), fitting the 2-sync TT encoding
                    dsb = mp.tile([128, 1], F32, tag="dsb")
                    nc.vector.tensor_scalar_mul(dsb, sg[:, 0:1], 1.0)
                    actT = mp.tile([128, NT], BF16, tag=f"actT{pair}")
                    nc.vector.tensor_tensor(
                        out=actT, in0=sg, in1=ps_u, op=mybir.AluOpType.mult,
                    )
                    actTs.append(actT)
                for m in range(KH):
                    ps_o = pp.tile([128, NT], F32, tag="pso")
                    for k2 in range(KI):
                        nc.tensor.matmul(
                            out=ps_o,
                            lhsT=w2Ts[k2][:, m * 128 : (m + 1) * 128],
                            rhs=actTs[k2],
                            start=(k2 == 0), stop=(k2 == KI - 1),
                        )
                    t1 = op_.tile([128, NT], F32, tag="t1")
                    nc.vector.tensor_tensor(
                        out=t1, in0=ps_o, in1=grep_sb, op=mybir.AluOpType.mult,
                    )
                    t2 = op_.tile([128, NT], F32, tag="t2")
                    nc.vector.tensor_tensor(
                        out=t2, in0=xTs[m][:, c0 : c0 + NT], in1=zrep_sb,
                        op=mybir.AluOpType.mult,
                    )
                    osb = op_.tile([128, NT], F32, tag="osb")
                    nc.vector.tensor_tensor(
                        out=osb, in0=t1, in1=t2, op=mybir.AluOpType.add,
                    )
                    nc.sync.dma_start(
                        out=outT_d[m * 128 : (m + 1) * 128, c0 : c0 + NT], in_=osb,
                    )
    nc.finalize()
    return nc


_PROGRAM_CACHE: dict[str, bass.Bass] = {}


def _selector_row(row: int) -> np.ndarray:
    s = np.zeros((16, 128), np.float32)
    s[row, :] = 1.0
    return s


def kernel(hidden_states, router_w, correction_bias, w13, w2, **_):
    hidden_states = np.asarray(hidden_states, dtype=np.float32)
    router_w = np.asarray(router_w, dtype=np.float32)
    correction_bias = np.asarray(correction_bias, dtype=np.float32)
    w13 = np.asarray(w13, dtype=np.float32)
    w2 = np.asarray(w2, dtype=np.float32)

    bf = ml_dtypes.bfloat16
    xT = np.ascontiguousarray(hidden_states.T).astype(bf)  # [H, T]

    in_maps = []
    for e in range(NCORES):
        perm = [e] + [x for x in range(E) if x != e] + list(range(E, NE))
        rwT = np.ascontiguousarray(router_w[perm].T).astype(bf)  # [H, 12]
        cb = np.ascontiguousarray(correction_bias[perm][None, :]).astype(np.float32)
        in_maps.append(
            {
                "xT": xT,
                "w13T": np.ascontiguousarray(w13[e].T).astype(bf),  # [H, 2I]
                "w2T": np.ascontiguousarray(w2[e].T).astype(bf),  # [I, H]
                "rwT": rwT,
                "cbias": cb,
                "ident": np.eye(128, dtype=np.float32),
                "selg": _selector_row(0),
                "selz": _selector_row(12) * (1.0 if e == 0 else 0.0),
                "ones1": np.ones((1, 128), np.float32),
            }
        )

    if "dense" not in _PROGRAM_CACHE:
        _PROGRAM_CACHE["dense"] = _build_dense_program()
    nc = _PROGRAM_CACHE["dense"]

    res = run_bass_kernel_spmd(nc, in_maps, list(range(NCORES)))
    acc = np.zeros((H, T), np.float32)
    for r in res.results:
        acc += r["outT"]
    return np.ascontiguousarray(acc.T)


if __name__ == "__main__":
    rng = np.random.default_rng(0)
    h = rng.standard_normal((T, H), dtype=np.float32)
    rw = rng.standard_normal((NE, H), dtype=np.float32) * 0.02
    cb = rng.standard_normal((NE,), dtype=np.float32) * 0.01
    w13_ = rng.standard_normal((E, I2, H), dtype=np.float32) * 0.02
    w2_ = rng.standard_normal((E, H, I), dtype=np.float32) * 0.02
    out = kernel(h, rw, cb, w13_, w2_)
    print(out.shape, out.dtype, np.abs(out).mean())
